# revision 38
# baseline (speedup 1.0000x reference)
"""Batch-hard triplet loss (CrossCameraTripletLoss) on 8 TRN2 NeuronCores.

All-transposed mining, engine-balanced (v6):
  - Host: stable-sort rows by label; shard 1024 sorted anchors per core.
    Columns rotated by (a0-64) so all class windows live in the band
    region [0, 1152) - same SPMD program on every core.
  - TRANSPOSED path (all 64 column-blocks): psum[col_p, anchor] =
    2<xj,xi> via bf16 matmuls.  The scalar engine fuses the per-partition
    bias (-sqj) into f32->f16 copies; non-band columns are sorted by sq
    and placed so PAIRS of blocks share one 2048-wide activation (adjacent
    sq ranks at the same partition -> shared midpoint bias, err <= 0.02).
    DVE chains tiles with 2x-rate f16 tensor_tensor max.  Band blocks
    chain with their window-polluted anchor interval SKIPPED.
  - BAND row-orient batch (positives + masked negatives over the skipped
    cells): one [128, 8, 256] psum; bias AND the +/-32768 window mask are
    folded into a rank-<=38 aux matmul (class-indicator outer products),
    so DVE only runs one pool_max and one reduce_min over it.
  - Finals: gpsimd partition_all_reduce, 8 tiny PE transposes, tail in
    [128, 8] layout, one scalar out per core; host sums / divides.
"""

import sys

sys.path.insert(0, "/opt/trn_rl_repo")

import numpy as np
import ml_dtypes

import concourse.bacc as bacc
import concourse.bass_isa as bass_isa
import concourse.mybir as mybir
import concourse.tile as tile
from concourse.bass_utils import run_bass_kernel_spmd

F32 = mybir.dt.float32
F16 = mybir.dt.float16
BF16 = mybir.dt.bfloat16
BF = ml_dtypes.bfloat16
H16 = np.float16
NEG_INF = -3.0e38
F16_LO = -60000.0
MARGIN = 0.2
BIG = 32768.0

N, D, NCORES = 8192, 128, 8
M = N // NCORES          # anchors per core
RB = M // 128            # row blocks per core
BW = 256                 # band width per row block
BOFF = 64                # rotation offset
NBAND = 9                # band jblocks ([0, 1152))
NJB = N // 128           # 64 total jblocks
NTR = NJB - NBAND        # 55 sq-sorted jblocks
NPAIR = NTR // 2         # 27 full non-band pairs (+1 single)
KAUX = 40                # padded aux rank for band mask matmul

TRACE = False
LAST_RESULTS = {}


def _skip_interval(jb):
    """Anchor interval polluted by band jblock jb (same-class pairs).
    Anchor t's window is rotated cols [t, t+129), so cols [128jb, 128jb+128)
    intersect windows of anchors [128jb-128, 128jb+128) exactly - which is
    precisely the region the masked band-row batch covers for those anchors."""
    lo = max(0, 128 * jb - 128)
    hi = min(M, 128 * jb + 128)
    return lo, hi


def _build_nc():
    nc = bacc.Bacc("TRN2", target_bir_lowering=False, debug=False)

    d_tfe = nc.dram_tensor("tfe", [D, N], BF16, kind="ExternalInput").ap()
    d_anch2 = nc.dram_tensor("anch2", [D, M], BF16, kind="ExternalInput").ap()
    d_brhs = nc.dram_tensor("brhs", [D, RB * BW], BF16, kind="ExternalInput").ap()
    d_balT = nc.dram_tensor("balT", [KAUX, M], BF16, kind="ExternalInput").ap()
    d_barhs = nc.dram_tensor("barhs", [KAUX, RB * BW], BF16, kind="ExternalInput").ap()
    d_prb = nc.dram_tensor("prb", [128, NPAIR], F32, kind="ExternalInput").ap()
    d_sb = nc.dram_tensor("sb", [128, 1], F32, kind="ExternalInput").ap()
    d_bb = nc.dram_tensor("bb", [128, NBAND], F32, kind="ExternalInput").ap()
    d_sqi = nc.dram_tensor("sqi", [128, RB], F32, kind="ExternalInput").ap()
    d_w = nc.dram_tensor("w", [128, RB], F32, kind="ExternalInput").ap()
    d_out = nc.dram_tensor("out", [1, 1], F32, kind="ExternalOutput").ap()

    AL = mybir.AluOpType
    AX = mybir.AxisListType
    AF = mybir.ActivationFunctionType

    with tile.TileContext(nc) as tc:
        with (
            tc.tile_pool(name="const", bufs=1) as const,
            tc.tile_pool(name="tps", bufs=2, space="PSUM") as tps,
            tc.tile_pool(name="tb", bufs=4) as tbp,
        ):
            t_tfe = const.tile([D, N], BF16)
            t_anch2 = const.tile([D, M], BF16)
            t_brhs = const.tile([D, RB * BW], BF16)
            t_balT = const.tile([KAUX, M], BF16)
            t_barhs = const.tile([KAUX, RB * BW], BF16)
            t_prb = const.tile([128, NPAIR], F32)
            t_sb = const.tile([128, 1], F32)
            t_bb = const.tile([128, NBAND], F32)
            t_sqi = const.tile([128, RB], F32)
            t_w = const.tile([128, RB], F32)
            t_one1 = const.tile([1, 1], F32)
            nc.vector.memset(t_one1[:], 1.0)

            # first consumers (pair units 0..2) need anch2 + tfe + prb first;
            # the band unit's inputs follow, then the tfe tail.
            nc.sync.dma_start(out=t_anch2[:], in_=d_anch2)
            nc.sync.dma_start(out=t_tfe[:, 0:2048], in_=d_tfe[:, 0:2048])
            nc.sync.dma_start(out=t_prb[:], in_=d_prb)
            nc.sync.dma_start(out=t_tfe[:, 2048:4096], in_=d_tfe[:, 2048:4096])
            for t, dr in [
                (t_brhs, d_brhs), (t_balT, d_balT), (t_barhs, d_barhs),
            ]:
                nc.sync.dma_start(out=t[:], in_=dr)
            for c in range(2, 4):
                sl = slice(c * 2048, (c + 1) * 2048)
                nc.sync.dma_start(out=t_tfe[:, sl], in_=d_tfe[:, sl])
            # finals-only inputs last - keeps the issue queue clear early
            for t, dr in [
                (t_bb, d_bb), (t_sb, d_sb), (t_sqi, d_sqi), (t_w, d_w),
            ]:
                nc.sync.dma_start(out=t[:], in_=dr)

            acc2 = const.tile([128, 2048], F16)
            nc.vector.memset(acc2[:], F16_LO)

            # ---- band row-orient batch: psum[p, rb, c] = 2dot - sqc - BIG*ind
            negband = const.tile([128, RB], F32)
            posmin = const.tile([128, RB], F32)

            def do_band_unit():
                psb = tps.tile([128, RB, BW], F32, tag="tps")
                for rb in range(RB):
                    bsl = slice(rb * BW, (rb + 1) * BW)
                    nc.tensor.matmul(
                        psb[:, rb, :],
                        lhsT=t_anch2[:, rb * 128:(rb + 1) * 128],
                        rhs=t_brhs[:, bsl],
                        start=True, stop=False,
                    )
                    nc.tensor.matmul(
                        psb[:, rb, :],
                        lhsT=t_balT[:, rb * 128:(rb + 1) * 128],
                        rhs=t_barhs[:, bsl],
                        start=False, stop=True,
                    )
                nc.vector.tensor_reduce(negband[:], psb[:], axis=AX.X, op=AL.max)
                nc.vector.tensor_reduce(posmin[:], psb[:], axis=AX.X, op=AL.min)

            # ---- transposed pair units ----
            # unit u covers jblocks (ja, jb_) in psum halves [0:1024),[1024:2048)
            def trans_unit(ja, jb2, bias_a, bias_b, one_act):
                pst = tps.tile([128, 2048], F32, tag="tps")
                for h, j in ((0, ja), (1, jb2)):
                    if j is None:
                        continue
                    for q in range(2):
                        sl = slice(h * 1024 + q * 512, h * 1024 + q * 512 + 512)
                        nc.tensor.matmul(
                            pst[:, sl],
                            lhsT=t_tfe[:, j * 128:(j + 1) * 128],
                            rhs=t_anch2[:, q * 512:(q + 1) * 512],
                            start=True, stop=True,
                        )
                tb = tbp.tile([128, 2048], F16, tag="tb")
                if one_act:
                    nc.scalar.activation(
                        tb[:], pst[:], AF.Identity, bias=bias_a, scale=1.0
                    )
                else:
                    # band halves: bias-copy on DVE to relieve the scalar
                    # engine (the sustained bottleneck)
                    nc.vector.tensor_scalar(
                        out=tb[:, 0:1024], in0=pst[:, 0:1024],
                        scalar1=bias_a, scalar2=None, op0=AL.add,
                    )
                    if jb2 is not None:
                        nc.vector.tensor_scalar(
                            out=tb[:, 1024:2048], in0=pst[:, 1024:2048],
                            scalar1=bias_b, scalar2=None, op0=AL.add,
                        )
                return tb

            def chain_full(tb):
                nc.vector.tensor_tensor(acc2[:], acc2[:], tb[:], AL.max)

            def chain_half(tb, h, skip=None):
                hsl_lo = h * 1024
                if skip is None:
                    nc.vector.tensor_tensor(
                        acc2[:, hsl_lo:hsl_lo + 1024],
                        acc2[:, hsl_lo:hsl_lo + 1024],
                        tb[:, hsl_lo:hsl_lo + 1024], AL.max,
                    )
                    return
                lo, hi = skip
                if lo > 0:
                    nc.vector.tensor_tensor(
                        acc2[:, hsl_lo:hsl_lo + lo],
                        acc2[:, hsl_lo:hsl_lo + lo],
                        tb[:, hsl_lo:hsl_lo + lo], AL.max,
                    )
                if hi < 1024:
                    nc.vector.tensor_tensor(
                        acc2[:, hsl_lo + hi:hsl_lo + 1024],
                        acc2[:, hsl_lo + hi:hsl_lo + 1024],
                        tb[:, hsl_lo + hi:hsl_lo + 1024], AL.max,
                    )

            # schedule: interleave band pairs among non-band pairs
            units = []
            for p in range(NPAIR):
                units.append(("pairs", p))
            for b in range(4):
                units.append(("bandpair", b))
            units.append(("mixed", None))
            # interleave: spread band units evenly
            order = []
            nb_i, b_i = 0, 0
            for u in range(32):
                if u % 7 == 6 and b_i < 5:
                    order.append(units[NPAIR + b_i]); b_i += 1
                elif nb_i < NPAIR:
                    order.append(units[nb_i]); nb_i += 1
                else:
                    order.append(units[NPAIR + b_i]); b_i += 1

            for ui, (kind, idx) in enumerate(order):
                if ui == 3:
                    # band unit after pipeline warm-up (its inputs arrive
                    # later and its consumers are DVE-only)
                    do_band_unit()
                if kind == "pairs":
                    ja, jb2 = NBAND + 2 * idx, NBAND + 2 * idx + 1
                    tb = trans_unit(ja, jb2, t_prb[:, idx:idx + 1], None, True)
                    chain_full(tb)
                elif kind == "bandpair":
                    ja, jb2 = 2 * idx, 2 * idx + 1
                    tb = trans_unit(
                        ja, jb2,
                        t_bb[:, ja:ja + 1], t_bb[:, jb2:jb2 + 1], False,
                    )
                    chain_half(tb, 0, _skip_interval(ja))
                    chain_half(tb, 1, _skip_interval(jb2))
                else:
                    # mixed: half A = non-band single (jb 63), half B = band jb 8
                    tb = trans_unit(
                        NJB - 1, 8, t_sb[:], t_bb[:, 8:9], False
                    )
                    chain_full_half = chain_half
                    chain_full_half(tb, 0, None)
                    chain_full_half(tb, 1, _skip_interval(8))

            # ---- finals ----
            accA = acc2[:, 0:1024]
            nc.vector.tensor_tensor(accA, accA, acc2[:, 1024:2048], AL.max)
            acc1 = const.tile([128, 1024], F16)
            nc.vector.tensor_copy(acc1[:], accA)
            tneg1 = const.tile([128, 1024], F32)
            nc.gpsimd.partition_all_reduce(
                tneg1[:], acc1[:], channels=128, reduce_op=bass_isa.ReduceOp.max
            )
            trps = tps.tile([128, 2048], F32, tag="tps")
            for rbk in range(RB):
                nc.tensor.transpose(
                    trps[:, rbk:rbk + 1],
                    tneg1[0:1, rbk * 128:(rbk + 1) * 128],
                    t_one1[:],
                )
            tnegT = const.tile([128, RB], F32)
            nc.vector.tensor_copy(tnegT[:], trps[:, 0:RB])

            negall = const.tile([128, RB], F32)
            nc.vector.tensor_tensor(negall[:], negband[:], tnegT[:], AL.max)

            # tail: d2pos = -posmin - BIG + sqi ; d2neg = sqi - negall
            d2p = const.tile([128, RB], F32)
            nc.vector.scalar_tensor_tensor(
                d2p[:], posmin[:], -1.0, t_sqi[:], op0=AL.mult, op1=AL.add
            )
            d2pb = const.tile([128, RB], F32)
            nc.vector.tensor_scalar_add(d2pb[:], d2p[:], -BIG)
            d2n = const.tile([128, RB], F32)
            nc.vector.tensor_tensor(d2n[:], t_sqi[:], negall[:], AL.subtract)
            rp = const.tile([128, RB], F32)
            rn = const.tile([128, RB], F32)
            nc.scalar.activation(rp[:], d2pb[:], AF.Relu)
            nc.scalar.activation(rn[:], d2n[:], AF.Relu)
            pd = const.tile([128, RB], F32)
            nd = const.tile([128, RB], F32)
            nc.scalar.activation(pd[:], rp[:], AF.Sqrt)
            nc.scalar.activation(nd[:], rn[:], AF.Sqrt)
            diff = const.tile([128, RB], F32)
            nc.vector.tensor_sub(diff[:], pd[:], nd[:])
            mgn = const.tile([128, 1], F32)
            nc.vector.memset(mgn[:], MARGIN)
            per = const.tile([128, RB], F32)
            nc.scalar.activation(per[:], diff[:], AF.Relu, bias=mgn[:])
            perw = const.tile([128, RB], F32)
            nc.vector.tensor_mul(perw[:], per[:], t_w[:])

            ones = const.tile([128, 1], F32)
            nc.vector.memset(ones[:], 1.0)
            sps = tps.tile([128, 2048], F32, tag="tps")
            nc.tensor.matmul(
                sps[0:1, 0:RB], lhsT=ones[:], rhs=perw[:], start=True, stop=True
            )
            srow = const.tile([1, RB], F32)
            nc.vector.tensor_copy(srow[:], sps[0:1, 0:RB])
            tot = const.tile([1, 1], F32)
            nc.vector.tensor_reduce(tot[:], srow[:], axis=AX.X, op=AL.add)
            nc.sync.dma_start(out=d_out, in_=tot[:])

    nc.compile()
    return nc


def _prep(features, labels):
    lab = np.asarray(labels).astype(np.int64).ravel()
    X = np.asarray(features, dtype=np.float32)
    assert X.shape == (N, D) and lab.shape == (N,)

    order = np.argsort(lab, kind="stable")
    Xs = np.ascontiguousarray(X[order])
    ls = lab[order]
    S = np.searchsorted(ls, ls, side="left").astype(np.int64)
    E = np.searchsorted(ls, ls, side="right").astype(np.int64)
    csize = E - S
    assert csize.max() <= BOFF + 1, f"class too large: {csize.max()}"
    valid = (csize < N).astype(np.float32)

    sq = (Xs.astype(np.float64) ** 2).sum(1).astype(np.float32)
    sq_hi = sq.astype(BF).astype(np.float32)
    sq_lo = sq - sq_hi
    XT = np.ascontiguousarray(Xs.T)                      # [D, N] f32

    in_maps = []
    total_valid = float(valid.sum())
    for k in range(NCORES):
        a0 = k * M
        colidx = (a0 - BOFF + np.arange(N)) % N          # rotated -> sorted
        bandcols = colidx[:NBAND * 128]                  # fixed order
        transcols = colidx[NBAND * 128:]                 # to be sq-sorted

        t_order = np.argsort(sq[transcols], kind="stable")
        tc = transcols[t_order]                          # 7040 cols by sq

        # placement: pair p partition q -> ranks 256p+2q (jb 2p), +1 (jb 2p+1)
        colof = np.empty(NJB * 128, np.int64)
        colof[:NBAND * 128] = bandcols
        for p in range(NPAIR):
            r0 = 256 * p
            colof[(NBAND + 2 * p) * 128:(NBAND + 2 * p + 1) * 128] = \
                tc[r0:r0 + 256:2]
            colof[(NBAND + 2 * p + 1) * 128:(NBAND + 2 * p + 2) * 128] = \
                tc[r0 + 1:r0 + 256:2]
        colof[(NJB - 1) * 128:] = tc[NPAIR * 256:]

        tfe = XT[:, colof].astype(BF)
        anch2 = (2.0 * XT[:, a0:a0 + M]).astype(BF)

        prb = np.empty((128, NPAIR), np.float32)
        for p in range(NPAIR):
            sa = sq[colof[(NBAND + 2 * p) * 128:(NBAND + 2 * p + 1) * 128]]
            sbq = sq[colof[(NBAND + 2 * p + 1) * 128:(NBAND + 2 * p + 2) * 128]]
            prb[:, p] = -0.5 * (sa + sbq)
        sb = (-sq[colof[(NJB - 1) * 128:]]).reshape(128, 1).astype(np.float32)
        bb = (-sq[bandcols]).reshape(NBAND, 128).T.astype(np.float32).copy()

        # band row-orient batch inputs
        brhs = np.empty((D, RB * BW), np.float32)
        balT = np.zeros((KAUX, M), np.float32)
        barhs = np.zeros((KAUX, RB * BW), np.float32)
        for rb in range(RB):
            bcols = colidx[128 * rb:128 * rb + BW]       # sorted idx of band
            bsl = slice(rb * BW, (rb + 1) * BW)
            brhs[:, bsl] = XT[:, bcols]      # unscaled: lhsT anch2 carries the 2x
            aidx = a0 + rb * 128 + np.arange(128)
            balT[0, rb * 128:(rb + 1) * 128] = 1.0
            balT[1, rb * 128:(rb + 1) * 128] = 1.0
            barhs[0, bsl] = -sq_hi[bcols]
            barhs[1, bsl] = -sq_lo[bcols]
            acl = ls[aidx]
            ucls = np.unique(acl)
            assert len(ucls) <= KAUX - 2, f"too many classes: {len(ucls)}"
            for ci, cval in enumerate(ucls):
                balT[2 + ci, rb * 128:(rb + 1) * 128] = (acl == cval)
                barhs[2 + ci, bsl] = -BIG * (ls[bcols] == cval)

        sqi = sq[a0:a0 + M].reshape(RB, 128).T.astype(np.float32).copy()
        w = valid[a0:a0 + M].reshape(RB, 128).T.astype(np.float32).copy()

        in_maps.append({
            "tfe": tfe, "anch2": anch2, "brhs": brhs.astype(BF),
            "balT": balT.astype(BF), "barhs": barhs.astype(BF),
            "prb": prb, "sb": sb, "bb": bb, "sqi": sqi, "w": w,
        })
    return in_maps, total_valid


_NC_CACHE = None


def kernel(features, labels):
    global _NC_CACHE, LAST_RESULTS
    in_maps, total_valid = _prep(features, labels)
    if _NC_CACHE is None:
        _NC_CACHE = _build_nc()
    nc = _NC_CACHE
    res = run_bass_kernel_spmd(nc, in_maps, list(range(NCORES)), trace=TRACE)
    LAST_RESULTS = {"bass": res}
    s = sum(float(res.results[k]["out"][0, 0]) for k in range(NCORES))
    loss = s / total_valid if total_valid > 0 else 0.0
    return np.float32(loss)


if __name__ == "__main__":
    from concourse.bass_interp import CoreSim

    sys.path.insert(0, "/root/problem")
    import reference

    inputs = {k: np.asarray(v) for k, v in reference.setup_inputs().items()}
    in_maps, total_valid = _prep(inputs["features"], inputs["labels"])
    nc = _build_nc()
    core = int(sys.argv[1]) if len(sys.argv) > 1 else 0
    sim = CoreSim(nc)
    for k2, v in in_maps[core].items():
        sim.tensor(k2)[:] = v
    sim.simulate()
    got = float(np.array(sim.tensor("out"))[0, 0])

    lab = np.asarray(inputs["labels"]).astype(np.int64).ravel()
    X = np.asarray(inputs["features"], np.float32)
    order = np.argsort(lab, kind="stable")
    Xs, ls = X[order], lab[order]
    d2 = ((Xs[core * M:(core + 1) * M, None] - Xs[None, :, :]) ** 2).sum(-1)
    pos_mask = ls[None, :] == ls[core * M:(core + 1) * M, None]
    pm = np.where(pos_mask, d2, -np.inf).max(1)
    nm = np.where(~pos_mask, d2, np.inf).min(1)
    per = np.maximum(np.sqrt(np.maximum(pm, 0)) - np.sqrt(np.maximum(nm, 0)) + MARGIN, 0)
    expected = per.sum()
    print(f"core{core} partial: got {got:.6f} expected {expected:.6f} "
          f"rel {abs(got - expected) / max(abs(expected), 1e-9):.3e}")
